# revision 13
# baseline (speedup 1.0000x reference)
"""GQA (B=2,S=1024,E=4096,H=32,KV=8,HD=128, RoPE, no causal mask) on 8 NeuronCores.

Sharding: 2 batch-groups x 4-way head tensor-parallel.
Core c: batch b=c//4, tp rank r=c%4 -> 8 q heads [8r,8r+8), 2 kv heads [2r,2r+2),
wo rows [1024r, 1024(r+1)).  Each core computes a partial output
out_part = y_local @ wo[local_rows, :]  (emitted transposed as [4096, 1024]);
host sums the 4 partials per batch. No device collectives needed.
"""
import sys

sys.path.insert(0, "/opt/trn_rl_repo")

import numpy as np

B = 2
S = 1024
E = 4096
HD = 128
N_CORES = 8
TP = 4            # tensor-parallel ranks per batch group
HL = 8            # q heads per core
KVL = 2           # kv heads per core
QCOLS = HL * HD   # 1024
KVCOLS = KVL * HD  # 256
NCC = (QCOLS + 2 * KVCOLS) // 128  # 12 col-chunks of 128 (8 q, 2 k, 2 v)
ECH = E // 128    # 32 e-chunks
TT = S // 128     # 8 token tiles
SCALE = 1.0 / np.sqrt(np.float32(HD))
MM_DT = "float16"   # matmul operand dtype: "float16" or "float32r"


_PROGRAM = None


def _build_program():
    import concourse.bass as bass  # noqa: F401
    from concourse import bacc
    import concourse.mybir as mybir
    from concourse.tile import TileContext
    from concourse.masks import make_identity

    dt = mybir.dt.float32
    dtr = getattr(mybir.dt, MM_DT)
    nc = bacc.Bacc("TRN2", target_bir_lowering=False, debug=False,
                   num_devices=N_CORES)

    xt_d = nc.declare_dram_parameter("xt", [E, S], dtr, isOutput=False)
    wq_d = nc.declare_dram_parameter("wq", [E, QCOLS], dtr, isOutput=False)
    wk_d = nc.declare_dram_parameter("wk", [E, KVCOLS], dtr, isOutput=False)
    wv_d = nc.declare_dram_parameter("wv", [E, KVCOLS], dtr, isOutput=False)
    wo_d = nc.declare_dram_parameter("wo", [QCOLS, E], dtr, isOutput=False)
    cos_d = nc.declare_dram_parameter("cos", [HD, S], dtr, isOutput=False)
    sinp_d = nc.declare_dram_parameter("sinp", [HD, S], dtr, isOutput=False)
    out_d = nc.declare_dram_parameter("out_t", [E, S], dt, isOutput=True)

    with TileContext(nc) as tc:
        with tc.tile_pool(name="const", bufs=1) as cpool, \
             tc.tile_pool(name="persist", bufs=1) as ppool, \
             tc.tile_pool(name="vnat", bufs=1) as vpool:
            ident_f = cpool.tile([128, 128], dt)
            make_identity(nc, ident_f[:])
            ident = cpool.tile([128, 128], dtr)
            nc.scalar.copy(ident[:], ident_f[:])
            ones_f = cpool.tile([128, 1], dt)
            nc.vector.memset(ones_f[:], 1.0)
            cos_t = cpool.tile([HD, S], dtr, tag="cos")
            sinp_t = cpool.tile([HD, S], dtr, tag="sinp")
            # persistent tiles: qkvT[cc] = [128 cols, S] transposed projections
            qkvT = [ppool.tile([128, S], dtr, tag=f"qkvT{i}", name=f"qkvT{i}") for i in range(NCC)]
            # yT[h] = [128 hd, S] transposed attention outputs
            yT = [ppool.tile([128, S], dtr, tag=f"yT{i}", name=f"yT{i}") for i in range(HL)]
            # v natural tiles with ones column: [128 k-tokens, HD+1]
            v_nat = [[vpool.tile([128, HD + 1], dtr, tag=f"v{kv}_{kt}", name=f"v{kv}_{kt}")
                      for kt in range(TT)] for kv in range(KVL)]

            # ---------------- Phase A: QKV^T projections (x^T from host) ----------------
            ECS = 8   # e-chunks per superchunk
            NSUP = ECH // ECS  # 4
            with tc.tile_pool(name="xsup", bufs=2) as xspool, \
                 tc.tile_pool(name="wstream", bufs=3) as wpool, \
                 tc.tile_pool(name="rope", bufs=3) as ropool, \
                 tc.tile_pool(name="psA", bufs=2, space="PSUM") as psA:
                CC_ORDER = [HL, HL + 1] + list(range(HL)) + [HL + KVL, HL + KVL + 1]

                def w_src(es, cc):
                    if cc < HL:
                        return wq_d[es * 1024:(es + 1) * 1024, cc * 128:(cc + 1) * 128]
                    if cc < HL + KVL:
                        return wk_d[es * 1024:(es + 1) * 1024,
                                    (cc - HL) * 128:(cc - HL + 1) * 128]
                    return wv_d[es * 1024:(es + 1) * 1024,
                                (cc - HL - KVL) * 128:(cc - HL - KVL + 1) * 128]

                for es in range(NSUP):
                    # first weight tile before the xs chunks so the first matmul
                    # is not stuck behind 8 queued DMAs
                    wt0 = wpool.tile([128, ECS, 128], dtr, tag="w", name=f"wt0_{es}")
                    nc.sync.dma_start(
                        out=wt0[:],
                        in_=w_src(es, CC_ORDER[0]).rearrange("(c p) m -> p c m", p=128))
                    xs = xspool.tile([128, ECS, S], dtr, tag="xs", name=f"xs{es}")
                    for ec in range(ECS):
                        base = es * 1024 + ec * 128
                        nc.sync.dma_start(
                            out=xs[:, ec, :], in_=xt_d[base:base + 128, :])
                    if es == 1:
                        nc.sync.dma_start(out=cos_t[:], in_=cos_d[:])
                        nc.sync.dma_start(out=sinp_t[:], in_=sinp_d[:])
                    for ci, cc in enumerate(CC_ORDER):
                        if ci == 0:
                            wt = wt0
                        else:
                            wt = wpool.tile([128, ECS, 128], dtr, tag="w")
                            nc.sync.dma_start(
                                out=wt[:],
                                in_=w_src(es, cc).rearrange("(c p) m -> p c m", p=128))
                        acc = psA.tile([128, S], dt, tag="acc")
                        for ec in range(ECS):
                            for tb in range(2):
                                nc.tensor.matmul(
                                    acc[:, tb * 512:(tb + 1) * 512], wt[:, ec, :],
                                    xs[:, ec, tb * 512:(tb + 1) * 512],
                                    start=(ec == 0), stop=(ec == ECS - 1))
                        if es == 0:
                            nc.scalar.copy(qkvT[cc][:], acc[:])
                        else:
                            nc.vector.tensor_add(qkvT[cc][:], acc[:], qkvT[cc][:])
                        if es == NSUP - 1 and cc < HL + KVL:
                            # rope immediately after the final accumulation of
                            # this chunk, overlapping remaining projections
                            sh = ropool.tile([HD, S], dtr, tag="sh")
                            nc.sync.dma_start(out=sh[0:64, :], in_=qkvT[cc][64:128, :])
                            nc.sync.dma_start(out=sh[64:128, :], in_=qkvT[cc][0:64, :])
                            t1 = ropool.tile([HD, S], dtr, tag="t1")
                            nc.vector.tensor_mul(t1[:], qkvT[cc][:], cos_t[:])
                            nc.vector.tensor_mul(sh[:], sh[:], sinp_t[:])
                            nc.vector.tensor_add(qkvT[cc][:], t1[:], sh[:])

            # ---------------- Phase C: V natural + ones column ----------------
            with tc.tile_pool(name="psC", bufs=2, space="PSUM") as psC:
                for kv in range(KVL):
                    for kt in range(TT):
                        pt = psC.tile([128, 128], dtr, tag="ptC")
                        nc.tensor.transpose(
                            pt[:], qkvT[HL + KVL + kv][:, kt * 128:(kt + 1) * 128], ident[:])
                        nc.vector.tensor_copy(v_nat[kv][kt][:, 0:HD], pt[:])
                        nc.vector.tensor_copy(v_nat[kv][kt][:, HD:HD + 1], ones_f[:])

            # ---------------- Phase D: attention per head ----------------
            with tc.tile_pool(name="pT", bufs=12) as ptpool, \
                 tc.tile_pool(name="ynorm", bufs=3) as ypool, \
                 tc.tile_pool(name="recs", bufs=3) as recpool, \
                 tc.tile_pool(name="psS", bufs=6, space="PSUM") as psS, \
                 tc.tile_pool(name="psY", bufs=1, space="PSUM") as psY, \
                 tc.tile_pool(name="psYT", bufs=1, space="PSUM") as psYT:
                for h in range(HL):
                    kv = h // (HL // KVL)
                    kT = qkvT[HL + kv]
                    pts = []
                    for kc in range(TT):
                        pt = ptpool.tile([128, S], dtr, tag="pT")
                        for tb in range(2):
                            sp = psS.tile([128, 512], dt, tag="sp")
                            nc.tensor.matmul(
                                sp[:],
                                kT[:, kc * 128:(kc + 1) * 128],
                                qkvT[h][:, tb * 512:(tb + 1) * 512],
                                start=True, stop=True)
                            nc.scalar.activation(pt[:, tb * 512:(tb + 1) * 512], sp[:],
                                                 mybir.ActivationFunctionType.Exp,
                                                 scale=float(SCALE))
                        pts.append(pt)
                    for qt in range(TT):
                        yp = psY.tile([128, HD + 1], dt, tag="yp")
                        for kc in range(TT):
                            nc.tensor.matmul(
                                yp[:], pts[kc][:, qt * 128:(qt + 1) * 128],
                                v_nat[kv][kc][:],
                                start=(kc == 0), stop=(kc == TT - 1))
                        rec = recpool.tile([128, 1], dt, tag="rec")
                        nc.vector.reciprocal(rec[:], yp[:, HD:HD + 1])
                        ysb = ypool.tile([128, HD], dtr, tag="ysb")
                        nc.vector.tensor_scalar_mul(ysb[:], yp[:, 0:HD], rec[:])
                        ytp = psYT.tile([128, 128], dtr, tag="ytp")
                        nc.tensor.transpose(ytp[:], ysb[:], ident[:])
                        nc.vector.tensor_copy(yT[h][:, qt * 128:(qt + 1) * 128], ytp[:])

            # ---------------- Phase E: out projection (partial, transposed) ----------------
            with tc.tile_pool(name="wo", bufs=3) as wopool, \
                 tc.tile_pool(name="osb", bufs=3) as opool, \
                 tc.tile_pool(name="psO", bufs=2, space="PSUM") as psO:
                for oc in range(E // 128):
                    op = psO.tile([128, S], dt, tag="op")
                    wt = wopool.tile([128, HL, 128], dtr, tag="wo")
                    nc.sync.dma_start(
                        out=wt[:],
                        in_=wo_d[:, oc * 128:(oc + 1) * 128].rearrange(
                            "(c p) m -> p c m", p=128))
                    for yc in range(HL):
                        for tb in range(2):
                            nc.tensor.matmul(
                                op[:, tb * 512:(tb + 1) * 512], wt[:, yc, :],
                                yT[yc][:, tb * 512:(tb + 1) * 512],
                                start=(yc == 0), stop=(yc == HL - 1))
                    ot = opool.tile([128, S], dt, tag="ot")
                    nc.scalar.copy(ot[:], op[:])
                    nc.sync.dma_start(
                        out=out_d[oc * 128:(oc + 1) * 128, :], in_=ot[:])

    nc.compile()
    return nc


def _rope_tables():
    inv = 1.0 / (10000.0 ** (np.arange(0, HD, 2, dtype=np.float32) / HD))  # [64]
    ang = np.arange(S, dtype=np.float32)[None, :] * inv[:, None]           # [64, S]
    cos = np.concatenate([np.cos(ang), np.cos(ang)], axis=0).astype(np.float32)   # [128, S]
    sin = np.sin(ang)
    sinp = np.concatenate([-sin, sin], axis=0).astype(np.float32)          # [128, S]
    return cos, sinp


def kernel(x, wq, wk, wv, wo):
    global _PROGRAM
    from concourse.bass_utils import run_bass_kernel_spmd

    if _PROGRAM is None:
        _PROGRAM = _build_program()
    nc = _PROGRAM

    cos, sinp = _rope_tables()
    ndt = np.float16 if MM_DT == "float16" else np.float32
    x = np.ascontiguousarray(x, dtype=np.float32)
    in_maps = []
    for c in range(N_CORES):
        b, r = c // TP, c % TP
        in_maps.append({
            "xt": np.ascontiguousarray(x[b].T).astype(ndt),
            "wq": np.ascontiguousarray(wq[:, r * QCOLS:(r + 1) * QCOLS], dtype=ndt),
            "wk": np.ascontiguousarray(wk[:, r * KVCOLS:(r + 1) * KVCOLS], dtype=ndt),
            "wv": np.ascontiguousarray(wv[:, r * KVCOLS:(r + 1) * KVCOLS], dtype=ndt),
            "wo": np.ascontiguousarray(wo[r * QCOLS:(r + 1) * QCOLS, :], dtype=ndt),
            "cos": cos.astype(ndt),
            "sinp": sinp.astype(ndt),
        })

    res = run_bass_kernel_spmd(nc, in_maps, list(range(N_CORES)))

    out = np.zeros((B, S, E), dtype=np.float32)
    for c in range(N_CORES):
        b = c // TP
        out[b] += res.results[c]["out_t"].T
    return out


# revision 14
# speedup vs baseline: 1.0570x; 1.0570x over previous
"""GQA (B=2,S=1024,E=4096,H=32,KV=8,HD=128, RoPE, no causal mask) on 8 NeuronCores.

Sharding: 2 batch-groups x 4-way head tensor-parallel.
Core c: batch b=c//4, tp rank r=c%4 -> 8 q heads [8r,8r+8), 2 kv heads [2r,2r+2),
wo rows [1024r, 1024(r+1)).  Each core computes a partial output
out_part = y_local @ wo[local_rows, :]  (emitted transposed as [4096, 1024]);
host sums the 4 partials per batch. No device collectives needed.
"""
import sys

sys.path.insert(0, "/opt/trn_rl_repo")

import numpy as np

B = 2
S = 1024
E = 4096
HD = 128
N_CORES = 8
TP = 4            # tensor-parallel ranks per batch group
HL = 8            # q heads per core
KVL = 2           # kv heads per core
QCOLS = HL * HD   # 1024
KVCOLS = KVL * HD  # 256
NCC = (QCOLS + 2 * KVCOLS) // 128  # 12 col-chunks of 128 (8 q, 2 k, 2 v)
ECH = E // 128    # 32 e-chunks
TT = S // 128     # 8 token tiles
SCALE = 1.0 / np.sqrt(np.float32(HD))
MM_DT = "float16"   # matmul operand dtype: "float16" or "float32r"


_PROGRAM = None


def _build_program():
    import concourse.bass as bass  # noqa: F401
    from concourse import bacc
    import concourse.mybir as mybir
    from concourse.tile import TileContext
    from concourse.masks import make_identity

    dt = mybir.dt.float32
    dtr = getattr(mybir.dt, MM_DT)
    nc = bacc.Bacc("TRN2", target_bir_lowering=False, debug=False,
                   num_devices=N_CORES)

    xt_d = nc.declare_dram_parameter("xt", [E, S], dtr, isOutput=False)
    wq_d = nc.declare_dram_parameter("wq", [E, QCOLS], dtr, isOutput=False)
    wk_d = nc.declare_dram_parameter("wk", [E, KVCOLS], dtr, isOutput=False)
    wv_d = nc.declare_dram_parameter("wv", [E, KVCOLS], dtr, isOutput=False)
    wo_d = nc.declare_dram_parameter("wo", [QCOLS, E], dtr, isOutput=False)
    cos_d = nc.declare_dram_parameter("cos", [HD, S], dtr, isOutput=False)
    sinp_d = nc.declare_dram_parameter("sinp", [HD, S], dtr, isOutput=False)
    out_d = nc.declare_dram_parameter("out_t", [E, S], dt, isOutput=True)

    with TileContext(nc) as tc:
        with tc.tile_pool(name="const", bufs=1) as cpool, \
             tc.tile_pool(name="persist", bufs=1) as ppool, \
             tc.tile_pool(name="vnat", bufs=1) as vpool:
            ident_f = cpool.tile([128, 128], dt)
            make_identity(nc, ident_f[:])
            ident = cpool.tile([128, 128], dtr)
            nc.scalar.copy(ident[:], ident_f[:])
            ones_f = cpool.tile([128, 1], dt)
            nc.vector.memset(ones_f[:], 1.0)
            cos_t = cpool.tile([HD, S], dtr, tag="cos")
            sinp_t = cpool.tile([HD, S], dtr, tag="sinp")
            # persistent tiles: qkvT[cc] = [128 cols, S] transposed projections
            qkvT = [ppool.tile([128, S], dtr, tag=f"qkvT{i}", name=f"qkvT{i}") for i in range(NCC)]
            # yT[h] = [128 hd, S] transposed attention outputs
            yT = [ppool.tile([128, S], dtr, tag=f"yT{i}", name=f"yT{i}") for i in range(HL)]
            # v natural tiles with ones column: [128 k-tokens, HD+1]
            v_nat = [[vpool.tile([128, HD + 1], dtr, tag=f"v{kv}_{kt}", name=f"v{kv}_{kt}")
                      for kt in range(TT)] for kv in range(KVL)]

            # ---------------- Phase A: QKV^T projections (x^T from host) ----------------
            ECS = 8   # e-chunks per superchunk
            NSUP = ECH // ECS  # 4
            with tc.tile_pool(name="xsup", bufs=2) as xspool, \
                 tc.tile_pool(name="wstream", bufs=3) as wpool, \
                 tc.tile_pool(name="rope", bufs=3) as ropool, \
                 tc.tile_pool(name="psA", bufs=2, space="PSUM") as psA:
                CC_ORDER = [HL, HL + 1] + list(range(HL)) + [HL + KVL, HL + KVL + 1]

                def w_src(es, cc):
                    if cc < HL:
                        return wq_d[es * 1024:(es + 1) * 1024, cc * 128:(cc + 1) * 128]
                    if cc < HL + KVL:
                        return wk_d[es * 1024:(es + 1) * 1024,
                                    (cc - HL) * 128:(cc - HL + 1) * 128]
                    return wv_d[es * 1024:(es + 1) * 1024,
                                (cc - HL - KVL) * 128:(cc - HL - KVL + 1) * 128]

                for es in range(NSUP):
                    # first weight tile before the xs chunks so the first matmul
                    # is not stuck behind 8 queued DMAs
                    wt0 = wpool.tile([128, ECS, 128], dtr, tag="w", name=f"wt0_{es}")
                    nc.sync.dma_start(
                        out=wt0[:],
                        in_=w_src(es, CC_ORDER[0]).rearrange("(c p) m -> p c m", p=128))
                    xs = xspool.tile([128, ECS, S], dtr, tag="xs", name=f"xs{es}")
                    for ec in range(ECS):
                        base = es * 1024 + ec * 128
                        nc.sync.dma_start(
                            out=xs[:, ec, :], in_=xt_d[base:base + 128, :])
                    if es == 1:
                        nc.sync.dma_start(out=cos_t[:], in_=cos_d[:])
                        nc.sync.dma_start(out=sinp_t[:], in_=sinp_d[:])
                    for ci, cc in enumerate(CC_ORDER):
                        if ci == 0:
                            wt = wt0
                        else:
                            wt = wpool.tile([128, ECS, 128], dtr, tag="w")
                            nc.sync.dma_start(
                                out=wt[:],
                                in_=w_src(es, cc).rearrange("(c p) m -> p c m", p=128))
                        acc = psA.tile([128, S], dt, tag="acc")
                        for ec in range(ECS):
                            for tb in range(2):
                                nc.tensor.matmul(
                                    acc[:, tb * 512:(tb + 1) * 512], wt[:, ec, :],
                                    xs[:, ec, tb * 512:(tb + 1) * 512],
                                    start=(ec == 0), stop=(ec == ECS - 1))
                        if es == 0:
                            nc.scalar.copy(qkvT[cc][:], acc[:])
                        else:
                            nc.vector.tensor_add(qkvT[cc][:], acc[:], qkvT[cc][:])
                        if es == NSUP - 1 and cc < HL + KVL:
                            # rope immediately after the final accumulation of
                            # this chunk, overlapping remaining projections
                            sh = ropool.tile([HD, S], dtr, tag="sh")
                            nc.sync.dma_start(out=sh[0:64, :], in_=qkvT[cc][64:128, :])
                            nc.sync.dma_start(out=sh[64:128, :], in_=qkvT[cc][0:64, :])
                            t1 = ropool.tile([HD, S], dtr, tag="t1")
                            nc.vector.tensor_mul(t1[:], qkvT[cc][:], cos_t[:])
                            nc.vector.tensor_mul(sh[:], sh[:], sinp_t[:])
                            nc.vector.tensor_add(qkvT[cc][:], t1[:], sh[:])

            # ---------------- Phase C: V natural + ones column ----------------
            with tc.tile_pool(name="psC", bufs=2, space="PSUM") as psC:
                for kv in range(KVL):
                    for kt in range(TT):
                        pt = psC.tile([128, 128], dtr, tag="ptC")
                        nc.tensor.transpose(
                            pt[:], qkvT[HL + KVL + kv][:, kt * 128:(kt + 1) * 128], ident[:])
                        nc.vector.tensor_copy(v_nat[kv][kt][:, 0:HD], pt[:])
                        nc.vector.tensor_copy(v_nat[kv][kt][:, HD:HD + 1], ones_f[:])

            # ---------------- Phase D: attention per head ----------------
            with tc.tile_pool(name="pT", bufs=12) as ptpool, \
                 tc.tile_pool(name="ynorm", bufs=3) as ypool, \
                 tc.tile_pool(name="recs", bufs=3) as recpool, \
                 tc.tile_pool(name="psS", bufs=4, space="PSUM") as psS, \
                 tc.tile_pool(name="psY", bufs=2, space="PSUM") as psY, \
                 tc.tile_pool(name="psYT", bufs=2, space="PSUM") as psYT:
                for h in range(HL):
                    kv = h // (HL // KVL)
                    kT = qkvT[HL + kv]
                    pts = []
                    for kc in range(TT):
                        pt = ptpool.tile([128, S], dtr, tag="pT")
                        for tb in range(2):
                            sp = psS.tile([128, 512], dt, tag="sp")
                            nc.tensor.matmul(
                                sp[:],
                                kT[:, kc * 128:(kc + 1) * 128],
                                qkvT[h][:, tb * 512:(tb + 1) * 512],
                                start=True, stop=True)
                            nc.scalar.activation(pt[:, tb * 512:(tb + 1) * 512], sp[:],
                                                 mybir.ActivationFunctionType.Exp,
                                                 scale=float(SCALE))
                        pts.append(pt)
                    for qt in range(TT):
                        yp = psY.tile([128, HD + 1], dt, tag="yp")
                        for kc in range(TT):
                            nc.tensor.matmul(
                                yp[:], pts[kc][:, qt * 128:(qt + 1) * 128],
                                v_nat[kv][kc][:],
                                start=(kc == 0), stop=(kc == TT - 1))
                        rec = recpool.tile([128, 1], dt, tag="rec")
                        nc.vector.reciprocal(rec[:], yp[:, HD:HD + 1])
                        ysb = ypool.tile([128, HD], dtr, tag="ysb")
                        nc.vector.tensor_scalar_mul(ysb[:], yp[:, 0:HD], rec[:])
                        ytp = psYT.tile([128, 128], dtr, tag="ytp")
                        nc.tensor.transpose(ytp[:], ysb[:], ident[:])
                        nc.vector.tensor_copy(yT[h][:, qt * 128:(qt + 1) * 128], ytp[:])

            # ---------------- Phase E: out projection (partial, transposed) ----------------
            with tc.tile_pool(name="wo", bufs=3) as wopool, \
                 tc.tile_pool(name="osb", bufs=3) as opool, \
                 tc.tile_pool(name="psO", bufs=2, space="PSUM") as psO:
                for oc in range(E // 128):
                    op = psO.tile([128, S], dt, tag="op")
                    wt = wopool.tile([128, HL, 128], dtr, tag="wo")
                    nc.sync.dma_start(
                        out=wt[:],
                        in_=wo_d[:, oc * 128:(oc + 1) * 128].rearrange(
                            "(c p) m -> p c m", p=128))
                    for yc in range(HL):
                        for tb in range(2):
                            nc.tensor.matmul(
                                op[:, tb * 512:(tb + 1) * 512], wt[:, yc, :],
                                yT[yc][:, tb * 512:(tb + 1) * 512],
                                start=(yc == 0), stop=(yc == HL - 1))
                    ot = opool.tile([128, S], dt, tag="ot")
                    nc.scalar.copy(ot[:], op[:])
                    nc.sync.dma_start(
                        out=out_d[oc * 128:(oc + 1) * 128, :], in_=ot[:])

    nc.compile()
    return nc


def _rope_tables():
    inv = 1.0 / (10000.0 ** (np.arange(0, HD, 2, dtype=np.float32) / HD))  # [64]
    ang = np.arange(S, dtype=np.float32)[None, :] * inv[:, None]           # [64, S]
    cos = np.concatenate([np.cos(ang), np.cos(ang)], axis=0).astype(np.float32)   # [128, S]
    sin = np.sin(ang)
    sinp = np.concatenate([-sin, sin], axis=0).astype(np.float32)          # [128, S]
    return cos, sinp


def kernel(x, wq, wk, wv, wo):
    global _PROGRAM
    from concourse.bass_utils import run_bass_kernel_spmd

    if _PROGRAM is None:
        _PROGRAM = _build_program()
    nc = _PROGRAM

    cos, sinp = _rope_tables()
    ndt = np.float16 if MM_DT == "float16" else np.float32
    x = np.ascontiguousarray(x, dtype=np.float32)
    in_maps = []
    for c in range(N_CORES):
        b, r = c // TP, c % TP
        in_maps.append({
            "xt": np.ascontiguousarray(x[b].T).astype(ndt),
            "wq": np.ascontiguousarray(wq[:, r * QCOLS:(r + 1) * QCOLS], dtype=ndt),
            "wk": np.ascontiguousarray(wk[:, r * KVCOLS:(r + 1) * KVCOLS], dtype=ndt),
            "wv": np.ascontiguousarray(wv[:, r * KVCOLS:(r + 1) * KVCOLS], dtype=ndt),
            "wo": np.ascontiguousarray(wo[r * QCOLS:(r + 1) * QCOLS, :], dtype=ndt),
            "cos": cos.astype(ndt),
            "sinp": sinp.astype(ndt),
        })

    res = run_bass_kernel_spmd(nc, in_maps, list(range(N_CORES)))

    out = np.zeros((B, S, E), dtype=np.float32)
    for c in range(N_CORES):
        b = c // TP
        out[b] += res.results[c]["out_t"].T
    return out
